# revision 1
# baseline (speedup 1.0000x reference)
"""LBN layer kernel for Trainium2 (8 NeuronCores, pure data parallel).

Inputs: E, px, py, pz each [262144, 16] f32.
Output: [262144, 424] = concat(E, px, py, pz, cross_z, cross_x, cross_y)
where cross(a,b)[r, (i,j)] = a[r,i]*b[r,j] - a[r,j]*b[r,i] over the 120
upper-triangle pairs (i<j), for (a,b) in [(px,py),(py,pz),(pz,px)].

Per-core scheme (32768 rows/core):
- Row r_local = p*256 + s maps to SBUF partition p, stripe slot s, so both
  the load and store DMAs are fully contiguous per partition.
- Per tile (nb row-blocks per partition): the 4 inputs are DMA'd straight
  into the out tile (cols 0:64 of each 424-wide block); for each feature,
  an outer-product M[b,i,j] = a_i*b_j (one wide tensor_tensor) followed by
  15 compact subtracts cross(i,j) = M[i,j] - M[j,i] written directly into
  the triu-packed output columns.
"""

import os

import numpy as np

import concourse.bass as bass
import concourse.tile as tile
from concourse import bacc, bass_utils, mybir

F32 = mybir.dt.float32
N = 16
NPAIR = (N * (N - 1)) // 2  # 120
OUTW = 4 * N + 3 * NPAIR  # 424
TOTAL_ROWS = 262144
NCORES = 8
ROWS_PER_CORE = TOTAL_ROWS // NCORES  # 32768

FEAT = [("px", "py"), ("py", "pz"), ("pz", "px")]
COL = {"E": 0, "px": 16, "py": 32, "pz": 48}


def build_kernel(
    nrows=ROWS_PER_CORE,
    nb=16,
    mul_eng=("v", "v", "g"),
    sub_eng=("v", "g", "v"),
    obufs=4,
    mbufs=3,
):
    """Build the per-core Bass module. nb = row-blocks per partition per tile."""
    stripe = nrows // 128
    assert stripe % nb == 0
    nt = stripe // nb

    nc = bacc.Bacc("TRN2", target_bir_lowering=False, debug=False)
    ins = {
        k: nc.dram_tensor(k, [nrows, N], F32, kind="ExternalInput").ap()
        for k in ["E", "px", "py", "pz"]
    }
    out = nc.dram_tensor("out", [nrows, OUTW], F32, kind="ExternalOutput").ap()
    inv = {k: v.rearrange("(p s) f -> p s f", p=128) for k, v in ins.items()}
    outv = out.rearrange("(p s) f -> p s f", p=128)

    def eng(c):
        return {"v": nc.vector, "g": nc.gpsimd}[c]

    with tile.TileContext(nc) as tc:
        with (
            tc.tile_pool(name="ot", bufs=obufs) as opool,
            tc.tile_pool(name="mt", bufs=mbufs) as mpool,
        ):
            for t in range(nt):
                ot = opool.tile([128, OUTW * nb], F32)
                ov = ot[:].rearrange("p (b c) -> p b c", c=OUTW)
                for k in ["E", "px", "py", "pz"]:
                    nc.sync.dma_start(
                        out=ov[:, :, COL[k] : COL[k] + N],
                        in_=inv[k][:, t * nb : (t + 1) * nb, :],
                    )
                off0 = 4 * N
                for f, (an, bn) in enumerate(FEAT):
                    mt = mpool.tile([128, N * N * nb], F32)
                    mv = mt[:].rearrange("p (b c) -> p b c", c=N * N)
                    mm = mv.rearrange("p b (i j) -> p b i j", j=N)
                    a = ov[:, :, COL[an] : COL[an] + N]
                    b = ov[:, :, COL[bn] : COL[bn] + N]
                    aap = a.unsqueeze(3).broadcast_to([128, nb, N, N])
                    bap = b.unsqueeze(2).broadcast_to([128, nb, N, N])
                    eng(mul_eng[f]).tensor_tensor(mm, aap, bap, mybir.AluOpType.mult)
                    off = off0 + NPAIR * f
                    for i in range(N - 1):
                        w = N - 1 - i
                        u = mv[:, :, 17 * i + 1 : 17 * i + 1 + w]
                        lo = mv[:, :, 17 * i + 16 : 17 * i + 16 + 16 * (w - 1) + 1 : 16]
                        o = ov[:, :, off : off + w]
                        eng(sub_eng[f]).tensor_tensor(
                            o, u, lo, mybir.AluOpType.subtract
                        )
                        off += w
                nc.sync.dma_start(out=outv[:, t * nb : (t + 1) * nb, :], in_=ot[:])
    nc.compile()
    return nc


_NC_CACHE = {}


def get_nc():
    cfg = os.environ.get("LBN_CFG", "")
    if cfg not in _NC_CACHE:
        kw = {}
        if cfg:
            # e.g. LBN_CFG="nb=16;mul=vvg;sub=vvg;obufs=3;mbufs=2"
            for part in cfg.split(";"):
                k, _, v = part.partition("=")
                if k in ("nb", "obufs", "mbufs"):
                    kw[k] = int(v)
                elif k == "mul":
                    kw["mul_eng"] = tuple(v)
                elif k == "sub":
                    kw["sub_eng"] = tuple(v)
        _NC_CACHE[cfg] = build_kernel(**kw)
    return _NC_CACHE[cfg]


_RUNNER = None


def _get_runner():
    """Cached jitted shard_map executable over the 8 cores (one compile)."""
    global _RUNNER
    if _RUNNER is not None:
        return _RUNNER
    import jax
    from jax.experimental.shard_map import shard_map
    from jax.sharding import Mesh, PartitionSpec

    from concourse import bass2jax

    nc = get_nc()
    bass2jax.install_neuronx_cc_hook()
    part_name = nc.partition_id_tensor.name if nc.partition_id_tensor else None
    in_names, out_names, out_avals, zero_outs = [], [], [], []
    for alloc in nc.m.functions[0].allocations:
        if not isinstance(alloc, mybir.MemoryLocationSet):
            continue
        name = alloc.memorylocations[0].name
        if alloc.kind == "ExternalInput":
            if name != part_name:
                in_names.append(name)
        elif alloc.kind == "ExternalOutput":
            shape = tuple(alloc.tensor_shape)
            dtype = mybir.dt.np(alloc.dtype)
            out_names.append(name)
            out_avals.append(jax.core.ShapedArray(shape, dtype))
            zero_outs.append(np.zeros(shape, dtype))
    all_names = in_names + out_names + ([part_name] if part_name else [])

    def _body(*args):
        operands = list(args)
        if part_name is not None:
            operands.append(bass2jax.partition_id_tensor())
        return tuple(
            bass2jax._bass_exec_p.bind(
                *operands,
                out_avals=tuple(out_avals),
                in_names=tuple(all_names),
                out_names=tuple(out_names),
                lowering_input_output_aliases=(),
                sim_require_finite=True,
                sim_require_nnan=True,
                nc=nc,
            )
        )

    devices = jax.devices()[:NCORES]
    mesh = Mesh(np.array(devices), ("core",))
    specs = (PartitionSpec("core"),) * (len(in_names) + len(out_names))
    out_specs = (PartitionSpec("core"),) * len(out_names)
    sharded = jax.jit(
        shard_map(
            _body, mesh=mesh, in_specs=specs, out_specs=out_specs, check_rep=False
        ),
        keep_unused=True,
    )
    concat_zeros = [
        np.zeros((NCORES * z.shape[0], *z.shape[1:]), z.dtype) for z in zero_outs
    ]
    _RUNNER = (sharded, in_names, concat_zeros)
    return _RUNNER


def kernel(E, px, py, pz):
    arrs = {
        "E": np.ascontiguousarray(np.asarray(E, dtype=np.float32)),
        "px": np.ascontiguousarray(np.asarray(px, dtype=np.float32)),
        "py": np.ascontiguousarray(np.asarray(py, dtype=np.float32)),
        "pz": np.ascontiguousarray(np.asarray(pz, dtype=np.float32)),
    }
    try:
        sharded, in_names, concat_zeros = _get_runner()
        outs = sharded(*[arrs[n] for n in in_names], *concat_zeros)
        return np.asarray(outs[0])
    except Exception:
        # robust fallback: the reference SPMD runner path
        nc = get_nc()
        in_maps = []
        for c in range(NCORES):
            sl = slice(c * ROWS_PER_CORE, (c + 1) * ROWS_PER_CORE)
            in_maps.append({k: v[sl] for k, v in arrs.items()})
        res = bass_utils.run_bass_kernel_spmd(
            nc, in_maps, core_ids=list(range(NCORES))
        )
        return np.concatenate([r["out"] for r in res.results], axis=0)


if __name__ == "__main__":
    rng = np.random.default_rng(0)
    ins = {
        k: rng.standard_normal((TOTAL_ROWS, N), dtype=np.float32)
        for k in ["E", "px", "py", "pz"]
    }
    out = kernel(**ins)
    print("out", out.shape, out.dtype)



# revision 2
# speedup vs baseline: 11.3468x; 11.3468x over previous
"""LBN layer kernel for Trainium2 (8 NeuronCores, pure data parallel).

Inputs: E, px, py, pz each [262144, 16] f32.
Output: [262144, 424] = concat(E, px, py, pz, cross_z, cross_x, cross_y)
where cross(a,b)[r, (i,j)] = a[r,i]*b[r,j] - a[r,j]*b[r,i] over the 120
upper-triangle pairs (i<j), for (a,b) in [(px,py),(py,pz),(pz,px)].

Per-core scheme (32768 rows/core; row r = p*256 + s -> partition p, slot s):
- All tensor compute runs on DVE only: DVE+GpSimd elementwise ops serialize
  on the shared SBUF port (measured), so GpSimd offload is a net loss.
- ACT (scalar engine) pre-builds fp16 operand blocks from the f32 loads:
  A=(px,py,pz), B=(py,pz,px) casts, As/Bs one-element-shifted copies, Ad/Bd
  pair-duplicated copies. These give every product instruction 2-byte dtypes,
  innermost stride-1 APs, and 4B-aligned starts, which keeps the DVE in its
  2x packed perf mode (fp32/broadcast operands would drop it to 1x).
- Per tile: 30 DVE mults write upper/lower pair products a_i*b_j / a_j*b_i
  into an even-padded packed layout; 15 DVE subtracts write the packed triu
  cross features into the fp16 out tile; ACT copies E,px,py,pz passthrough.
- Store: SWDGE cast-DMA (fp16 SBUF -> f32 HBM rows, contiguous per
  partition); loads are plain HWDGE f32.
Measured ~201us/core/iteration vs ~195us for the DMA traffic alone
(63.6MB/core vs ~358GB/s HBM => ~177us roofline).
"""

import numpy as np

import concourse.bass as bass
import concourse.tile as tile
from concourse import bacc, bass_utils, mybir

F32 = mybir.dt.float32
F16 = mybir.dt.float16
N = 16
NPAIR = (N * (N - 1)) // 2  # 120
OUTW = 4 * N + 3 * NPAIR  # 424
TOTAL_ROWS = 262144
NCORES = 8
ROWS_PER_CORE = TOTAL_ROWS // NCORES  # 32768

TRIU_OFF = [0]
for _i in range(1, N):
    TRIU_OFF.append(TRIU_OFF[-1] + (N - _i))

WPAD = [(15 - i) + ((15 - i) & 1) for i in range(15)]  # even-padded widths
POFF = [0]
for _w in WPAD[:-1]:
    POFF.append(POFF[-1] + _w)
PADLEN = POFF[-1] + WPAD[-1]  # 128


def build_kernel(
    nrows=ROWS_PER_CORE,
    nb=16,
    repeat=1,
    share_out=True,
    xbufs=2,
    fbufs=2,
    ubufs=1,
    obufs=3,
):
    """Build the per-core Bass module.

    nb: row-block (per partition) per tile. repeat: number of full kernel
    executions in one NEFF (used by test.py for amortized HW timing; each
    repetition re-loads inputs from HBM and re-stores the full output).
    share_out: all repetitions write the same DRAM output (identical bytes).
    """
    stripe = nrows // 128
    assert stripe % nb == 0
    nt = stripe // nb

    nc = bacc.Bacc("TRN2", target_bir_lowering=False, debug=False)
    ins = {
        k: nc.dram_tensor(k, [nrows, N], F32, kind="ExternalInput").ap()
        for k in ["E", "px", "py", "pz"]
    }
    nout = 1 if share_out else repeat
    outs = [
        nc.dram_tensor(f"out{r}" if r else "out", [nrows, OUTW], F32,
                       kind="ExternalOutput").ap()
        for r in range(nout)
    ]
    inv = {k: v.rearrange("(p s) f -> p s f", p=128) for k, v in ins.items()}

    B3 = 3 * N  # elems (per nb) of one [3, nb, 16] fp16 block
    OFF_A, OFF_B, OFF_AS, OFF_BS, OFF_AD, OFF_BD = (
        0, B3, 2 * B3, 3 * B3, 4 * B3, 6 * B3)
    XFLEN = 8 * B3  # Ad/Bd are double width

    with tile.TileContext(nc) as tc:
        with (
            tc.tile_pool(name="xt", bufs=xbufs) as xpool,
            tc.tile_pool(name="xf", bufs=fbufs) as fpool,
            tc.tile_pool(name="ul", bufs=ubufs) as ulpool,
            tc.tile_pool(name="ot", bufs=obufs) as opool,
        ):
            for rep in range(repeat):
                outv = outs[0 if share_out else rep].rearrange(
                    "(p s) f -> p s f", p=128
                )
                for t in range(nt):
                    # f32 input tile [4, nb, 16] = E,px,py,pz (HWDGE loads)
                    xt = xpool.tile([128, 4 * N * nb], F32, tag="xt")
                    xk = xt[:].rearrange("p (k s i) -> p k s i", s=nb, i=N)
                    for kidx, k in enumerate(["E", "px", "py", "pz"]):
                        nc.sync.dma_start(
                            out=xk[:, kidx],
                            in_=inv[k][:, t * nb : (t + 1) * nb, :],
                        )

                    # fp16 operand tile, built on ACT
                    xf = fpool.tile([128, XFLEN * nb], F16, tag="xf")

                    def blk(off, ln):
                        return xf[:, off * nb : off * nb + ln * nb]

                    A, As = blk(OFF_A, B3), blk(OFF_AS, B3)
                    B, Bs = blk(OFF_B, B3), blk(OFF_BS, B3)
                    Ad, Bd = blk(OFF_AD, 2 * B3), blk(OFF_BD, 2 * B3)

                    nc.scalar.copy(out=A, in_=xt[:, N * nb : 4 * N * nb])
                    nc.scalar.copy(
                        out=B[:, : 2 * N * nb], in_=xt[:, 2 * N * nb : 4 * N * nb]
                    )
                    nc.scalar.copy(
                        out=B[:, 2 * N * nb :], in_=xt[:, N * nb : 2 * N * nb]
                    )
                    # shifted copies, full length: the final element reads the
                    # first element of the following (already written) block
                    nc.scalar.copy(
                        out=As, in_=xf[:, OFF_A * nb + 1 : OFF_A * nb + B3 * nb + 1]
                    )
                    nc.scalar.copy(
                        out=Bs, in_=xf[:, OFF_B * nb + 1 : OFF_B * nb + B3 * nb + 1]
                    )
                    # pair-duplicated copies [fs, i, 2] <- [fs, i] broadcast
                    Av = A.rearrange("p (fs i) -> p fs i", i=N)
                    Bv = B.rearrange("p (fs i) -> p fs i", i=N)
                    Adv = Ad.rearrange("p (fs i two) -> p fs i two", i=N, two=2)
                    Bdv = Bd.rearrange("p (fs i two) -> p fs i two", i=N, two=2)
                    nc.scalar.copy(
                        out=Adv, in_=Av.unsqueeze(3).broadcast_to([128, 3 * nb, N, 2])
                    )
                    nc.scalar.copy(
                        out=Bdv, in_=Bv.unsqueeze(3).broadcast_to([128, 3 * nb, N, 2])
                    )
                    Asv = As.rearrange("p (fs i) -> p fs i", i=N)
                    Bsv = Bs.rearrange("p (fs i) -> p fs i", i=N)

                    # padded u/l product tiles (DVE-only producer+consumer)
                    u3 = ulpool.tile([128, 3 * nb * PADLEN], F16, tag="u3")
                    l3 = ulpool.tile([128, 3 * nb * PADLEN], F16, tag="l3")
                    uq = u3[:].rearrange("p (fs q) -> p fs q", q=PADLEN)
                    lq = l3[:].rearrange("p (fs q) -> p fs q", q=PADLEN)

                    ot = opool.tile([128, nb * OUTW], F16, tag="ot")
                    ov = ot[:].rearrange("p (s c) -> p s c", c=OUTW)
                    pass_dst = ov[:, :, 0 : 4 * N].rearrange(
                        "p s (k i) -> p k s i", i=N
                    )
                    nc.scalar.copy(out=pass_dst, in_=xk[:, 0:4])

                    def pairs(view3, start, np2):
                        return (
                            view3[:, :, start : start + 2 * np2]
                            .rearrange("p fs (q two) -> p fs q two", two=2)
                        )

                    for i in range(15):
                        wp = WPAD[i]
                        q0 = POFF[i]
                        np2 = wp // 2
                        Adp = Adv[:, :, i, :].unsqueeze(2).broadcast_to(
                            [128, 3 * nb, np2, 2]
                        )
                        Bdp = Bdv[:, :, i, :].unsqueeze(2).broadcast_to(
                            [128, 3 * nb, np2, 2]
                        )
                        # window operand starting at element i+1: base block if
                        # i+1 is even, else the shifted block at i (even)
                        if (i + 1) % 2 == 0:
                            bwin = pairs(Bv, i + 1, np2)
                            awin = pairs(Av, i + 1, np2)
                        else:
                            bwin = pairs(Bsv, i, np2)
                            awin = pairs(Asv, i, np2)
                        uo = uq[:, :, q0 : q0 + wp].rearrange(
                            "p fs (q two) -> p fs q two", two=2
                        )
                        lo = lq[:, :, q0 : q0 + wp].rearrange(
                            "p fs (q two) -> p fs q two", two=2
                        )
                        nc.vector.tensor_tensor(uo, Adp, bwin, mybir.AluOpType.mult)
                        nc.vector.tensor_tensor(lo, awin, Bdp, mybir.AluOpType.mult)

                    # per-segment subtract into the packed triu out columns
                    uf = u3[:].rearrange("p (f s q) -> p f s q", s=nb, q=PADLEN)
                    lf = l3[:].rearrange("p (f s q) -> p f s q", s=nb, q=PADLEN)
                    crossv = ov[:, :, 4 * N : OUTW].rearrange(
                        "p s (f q) -> p f s q", q=NPAIR
                    )
                    for i in range(15):
                        w = 15 - i
                        q0 = POFF[i]
                        o0 = TRIU_OFF[i]
                        nc.vector.tensor_tensor(
                            crossv[:, :, :, o0 : o0 + w],
                            uf[:, :, :, q0 : q0 + w],
                            lf[:, :, :, q0 : q0 + w],
                            mybir.AluOpType.subtract,
                        )

                    # SWDGE cast store: fp16 SBUF -> f32 HBM
                    nc.gpsimd.dma_start(
                        out=outv[:, t * nb : (t + 1) * nb, :],
                        in_=ot[:].rearrange("p (s c) -> p s c", c=OUTW),
                    )
    nc.compile()
    return nc


_NC_CACHE = {}


def get_nc(**kw):
    key = tuple(sorted(kw.items()))
    if key not in _NC_CACHE:
        _NC_CACHE[key] = build_kernel(**kw)
    return _NC_CACHE[key]


_RUNNER = None


def _get_runner():
    """Cached jitted shard_map executable over the 8 cores (one compile)."""
    global _RUNNER
    if _RUNNER is not None:
        return _RUNNER
    import jax
    from jax.experimental.shard_map import shard_map
    from jax.sharding import Mesh, PartitionSpec

    from concourse import bass2jax

    nc = get_nc()
    bass2jax.install_neuronx_cc_hook()
    part_name = nc.partition_id_tensor.name if nc.partition_id_tensor else None
    in_names, out_names, out_avals, zero_outs = [], [], [], []
    for alloc in nc.m.functions[0].allocations:
        if not isinstance(alloc, mybir.MemoryLocationSet):
            continue
        name = alloc.memorylocations[0].name
        if alloc.kind == "ExternalInput":
            if name != part_name:
                in_names.append(name)
        elif alloc.kind == "ExternalOutput":
            shape = tuple(alloc.tensor_shape)
            dtype = mybir.dt.np(alloc.dtype)
            out_names.append(name)
            out_avals.append(jax.core.ShapedArray(shape, dtype))
            zero_outs.append(np.zeros(shape, dtype))
    all_names = in_names + out_names + ([part_name] if part_name else [])

    def _body(*args):
        operands = list(args)
        if part_name is not None:
            operands.append(bass2jax.partition_id_tensor())
        return tuple(
            bass2jax._bass_exec_p.bind(
                *operands,
                out_avals=tuple(out_avals),
                in_names=tuple(all_names),
                out_names=tuple(out_names),
                lowering_input_output_aliases=(),
                sim_require_finite=True,
                sim_require_nnan=True,
                nc=nc,
            )
        )

    devices = jax.devices()[:NCORES]
    mesh = Mesh(np.array(devices), ("core",))
    specs = (PartitionSpec("core"),) * (len(in_names) + len(out_names))
    out_specs = (PartitionSpec("core"),) * len(out_names)
    sharded = jax.jit(
        shard_map(
            _body, mesh=mesh, in_specs=specs, out_specs=out_specs, check_rep=False
        ),
        keep_unused=True,
    )
    concat_zeros = [
        np.zeros((NCORES * z.shape[0], *z.shape[1:]), z.dtype) for z in zero_outs
    ]
    _RUNNER = (sharded, in_names, concat_zeros)
    return _RUNNER


def kernel(E, px, py, pz):
    arrs = {
        "E": np.ascontiguousarray(np.asarray(E, dtype=np.float32)),
        "px": np.ascontiguousarray(np.asarray(px, dtype=np.float32)),
        "py": np.ascontiguousarray(np.asarray(py, dtype=np.float32)),
        "pz": np.ascontiguousarray(np.asarray(pz, dtype=np.float32)),
    }
    try:
        sharded, in_names, concat_zeros = _get_runner()
        outs = sharded(*[arrs[n] for n in in_names], *concat_zeros)
        return np.asarray(outs[0])
    except Exception:
        # robust fallback: the reference SPMD runner path
        nc = get_nc()
        in_maps = []
        for c in range(NCORES):
            sl = slice(c * ROWS_PER_CORE, (c + 1) * ROWS_PER_CORE)
            in_maps.append({k: v[sl] for k, v in arrs.items()})
        res = bass_utils.run_bass_kernel_spmd(
            nc, in_maps, core_ids=list(range(NCORES))
        )
        return np.concatenate([r["out"] for r in res.results], axis=0)


if __name__ == "__main__":
    rng = np.random.default_rng(0)
    ins = {
        k: rng.standard_normal((TOTAL_ROWS, N), dtype=np.float32)
        for k in ["E", "px", "py", "pz"]
    }
    out = kernel(**ins)
    print("out", out.shape, out.dtype)


# revision 3
# speedup vs baseline: 11.6556x; 1.0272x over previous
"""LBN layer kernel for Trainium2 (8 NeuronCores, pure data parallel).

Inputs: E, px, py, pz each [262144, 16] f32.
Output: [262144, 424] = concat(E, px, py, pz, cross_z, cross_x, cross_y)
where cross(a,b)[r, (i,j)] = a[r,i]*b[r,j] - a[r,j]*b[r,i] over the 120
upper-triangle pairs (i<j), for (a,b) in [(px,py),(py,pz),(pz,px)].

Per-core scheme (32768 rows/core; row r = p*256 + s -> partition p, slot s):
- All tensor compute runs on DVE only: DVE+GpSimd elementwise ops serialize
  on the shared SBUF port (measured), so GpSimd offload is a net loss.
- ACT (scalar engine) pre-builds fp16 operand blocks from the f32 loads:
  A=(px,py,pz), B=(py,pz,px) casts, As/Bs one-element-shifted copies, Ad/Bd
  pair-duplicated copies. These give every product instruction 2-byte dtypes,
  innermost stride-1 APs, and 4B-aligned starts, which keeps the DVE in its
  2x packed perf mode (fp32/broadcast operands would drop it to 1x).
- Per tile: 30 DVE mults write upper/lower pair products a_i*b_j / a_j*b_i
  into an even-padded packed layout; 15 DVE subtracts write the packed triu
  cross features into the fp16 out tile; ACT copies E,px,py,pz passthrough.
- Store: SWDGE cast-DMA (fp16 SBUF -> f32 HBM rows, contiguous per
  partition); loads are plain HWDGE f32.
Measured ~201us/core/iteration vs ~195us for the DMA traffic alone
(63.6MB/core vs ~358GB/s HBM => ~177us roofline).
"""

import numpy as np

import concourse.bass as bass
import concourse.tile as tile
from concourse import bacc, bass_utils, mybir

F32 = mybir.dt.float32
F16 = mybir.dt.float16
N = 16
NPAIR = (N * (N - 1)) // 2  # 120
OUTW = 4 * N + 3 * NPAIR  # 424
TOTAL_ROWS = 262144
NCORES = 8
ROWS_PER_CORE = TOTAL_ROWS // NCORES  # 32768

TRIU_OFF = [0]
for _i in range(1, N):
    TRIU_OFF.append(TRIU_OFF[-1] + (N - _i))

WPAD = [(15 - i) + ((15 - i) & 1) for i in range(15)]  # even-padded widths
POFF = [0]
for _w in WPAD[:-1]:
    POFF.append(POFF[-1] + _w)
PADLEN = POFF[-1] + WPAD[-1]  # 128


def build_kernel(
    nrows=ROWS_PER_CORE,
    nb=16,
    repeat=1,
    share_out=True,
    xbufs=3,
    fbufs=3,
    ubufs=1,
    obufs=4,
):
    """Build the per-core Bass module.

    nb: row-block (per partition) per tile. repeat: number of full kernel
    executions in one NEFF (used by test.py for amortized HW timing; each
    repetition re-loads inputs from HBM and re-stores the full output).
    share_out: all repetitions write the same DRAM output (identical bytes).
    """
    stripe = nrows // 128
    assert stripe % nb == 0
    nt = stripe // nb

    nc = bacc.Bacc("TRN2", target_bir_lowering=False, debug=False)
    ins = {
        k: nc.dram_tensor(k, [nrows, N], F32, kind="ExternalInput").ap()
        for k in ["E", "px", "py", "pz"]
    }
    nout = 1 if share_out else repeat
    outs = [
        nc.dram_tensor(f"out{r}" if r else "out", [nrows, OUTW], F32,
                       kind="ExternalOutput").ap()
        for r in range(nout)
    ]
    inv = {k: v.rearrange("(p s) f -> p s f", p=128) for k, v in ins.items()}

    B3 = 3 * N  # elems (per nb) of one [3, nb, 16] fp16 block
    OFF_A, OFF_B, OFF_AS, OFF_BS, OFF_AD, OFF_BD = (
        0, B3, 2 * B3, 3 * B3, 4 * B3, 6 * B3)
    XFLEN = 8 * B3  # Ad/Bd are double width

    with tile.TileContext(nc) as tc:
        with (
            tc.tile_pool(name="xt", bufs=xbufs) as xpool,
            tc.tile_pool(name="xf", bufs=fbufs) as fpool,
            tc.tile_pool(name="ul", bufs=ubufs) as ulpool,
            tc.tile_pool(name="ot", bufs=obufs) as opool,
        ):
            for rep in range(repeat):
                outv = outs[0 if share_out else rep].rearrange(
                    "(p s) f -> p s f", p=128
                )
                for t in range(nt):
                    # f32 input tile [4, nb, 16] = E,px,py,pz (HWDGE loads)
                    xt = xpool.tile([128, 4 * N * nb], F32, tag="xt")
                    xk = xt[:].rearrange("p (k s i) -> p k s i", s=nb, i=N)
                    for kidx, k in enumerate(["E", "px", "py", "pz"]):
                        nc.sync.dma_start(
                            out=xk[:, kidx],
                            in_=inv[k][:, t * nb : (t + 1) * nb, :],
                        )

                    # fp16 operand tile, built on ACT
                    xf = fpool.tile([128, XFLEN * nb], F16, tag="xf")

                    def blk(off, ln):
                        return xf[:, off * nb : off * nb + ln * nb]

                    A, As = blk(OFF_A, B3), blk(OFF_AS, B3)
                    B, Bs = blk(OFF_B, B3), blk(OFF_BS, B3)
                    Ad, Bd = blk(OFF_AD, 2 * B3), blk(OFF_BD, 2 * B3)

                    nc.scalar.copy(out=A, in_=xt[:, N * nb : 4 * N * nb])
                    nc.scalar.copy(
                        out=B[:, : 2 * N * nb], in_=xt[:, 2 * N * nb : 4 * N * nb]
                    )
                    nc.scalar.copy(
                        out=B[:, 2 * N * nb :], in_=xt[:, N * nb : 2 * N * nb]
                    )
                    # shifted copies, full length: the final element reads the
                    # first element of the following (already written) block
                    nc.scalar.copy(
                        out=As, in_=xf[:, OFF_A * nb + 1 : OFF_A * nb + B3 * nb + 1]
                    )
                    nc.scalar.copy(
                        out=Bs, in_=xf[:, OFF_B * nb + 1 : OFF_B * nb + B3 * nb + 1]
                    )
                    # pair-duplicated copies [fs, i, 2] <- [fs, i] broadcast
                    Av = A.rearrange("p (fs i) -> p fs i", i=N)
                    Bv = B.rearrange("p (fs i) -> p fs i", i=N)
                    Adv = Ad.rearrange("p (fs i two) -> p fs i two", i=N, two=2)
                    Bdv = Bd.rearrange("p (fs i two) -> p fs i two", i=N, two=2)
                    nc.scalar.copy(
                        out=Adv, in_=Av.unsqueeze(3).broadcast_to([128, 3 * nb, N, 2])
                    )
                    nc.scalar.copy(
                        out=Bdv, in_=Bv.unsqueeze(3).broadcast_to([128, 3 * nb, N, 2])
                    )
                    Asv = As.rearrange("p (fs i) -> p fs i", i=N)
                    Bsv = Bs.rearrange("p (fs i) -> p fs i", i=N)

                    # padded u/l product tiles (DVE-only producer+consumer)
                    u3 = ulpool.tile([128, 3 * nb * PADLEN], F16, tag="u3")
                    l3 = ulpool.tile([128, 3 * nb * PADLEN], F16, tag="l3")
                    uq = u3[:].rearrange("p (fs q) -> p fs q", q=PADLEN)
                    lq = l3[:].rearrange("p (fs q) -> p fs q", q=PADLEN)

                    ot = opool.tile([128, nb * OUTW], F16, tag="ot")
                    ov = ot[:].rearrange("p (s c) -> p s c", c=OUTW)
                    pass_dst = ov[:, :, 0 : 4 * N].rearrange(
                        "p s (k i) -> p k s i", i=N
                    )
                    nc.scalar.copy(out=pass_dst, in_=xk[:, 0:4])

                    def pairs(view3, start, np2):
                        return (
                            view3[:, :, start : start + 2 * np2]
                            .rearrange("p fs (q two) -> p fs q two", two=2)
                        )

                    for i in range(15):
                        wp = WPAD[i]
                        q0 = POFF[i]
                        np2 = wp // 2
                        Adp = Adv[:, :, i, :].unsqueeze(2).broadcast_to(
                            [128, 3 * nb, np2, 2]
                        )
                        Bdp = Bdv[:, :, i, :].unsqueeze(2).broadcast_to(
                            [128, 3 * nb, np2, 2]
                        )
                        # window operand starting at element i+1: base block if
                        # i+1 is even, else the shifted block at i (even)
                        if (i + 1) % 2 == 0:
                            bwin = pairs(Bv, i + 1, np2)
                            awin = pairs(Av, i + 1, np2)
                        else:
                            bwin = pairs(Bsv, i, np2)
                            awin = pairs(Asv, i, np2)
                        uo = uq[:, :, q0 : q0 + wp].rearrange(
                            "p fs (q two) -> p fs q two", two=2
                        )
                        lo = lq[:, :, q0 : q0 + wp].rearrange(
                            "p fs (q two) -> p fs q two", two=2
                        )
                        nc.vector.tensor_tensor(uo, Adp, bwin, mybir.AluOpType.mult)
                        nc.vector.tensor_tensor(lo, awin, Bdp, mybir.AluOpType.mult)

                    # per-segment subtract into the packed triu out columns
                    uf = u3[:].rearrange("p (f s q) -> p f s q", s=nb, q=PADLEN)
                    lf = l3[:].rearrange("p (f s q) -> p f s q", s=nb, q=PADLEN)
                    crossv = ov[:, :, 4 * N : OUTW].rearrange(
                        "p s (f q) -> p f s q", q=NPAIR
                    )
                    for i in range(15):
                        w = 15 - i
                        q0 = POFF[i]
                        o0 = TRIU_OFF[i]
                        nc.vector.tensor_tensor(
                            crossv[:, :, :, o0 : o0 + w],
                            uf[:, :, :, q0 : q0 + w],
                            lf[:, :, :, q0 : q0 + w],
                            mybir.AluOpType.subtract,
                        )

                    # SWDGE cast store: fp16 SBUF -> f32 HBM
                    nc.gpsimd.dma_start(
                        out=outv[:, t * nb : (t + 1) * nb, :],
                        in_=ot[:].rearrange("p (s c) -> p s c", c=OUTW),
                    )
    nc.compile()
    return nc


_NC_CACHE = {}


def get_nc(**kw):
    key = tuple(sorted(kw.items()))
    if key not in _NC_CACHE:
        _NC_CACHE[key] = build_kernel(**kw)
    return _NC_CACHE[key]


_RUNNER = None


def _get_runner():
    """Cached jitted shard_map executable over the 8 cores (one compile)."""
    global _RUNNER
    if _RUNNER is not None:
        return _RUNNER
    import jax
    from jax.experimental.shard_map import shard_map
    from jax.sharding import Mesh, PartitionSpec

    from concourse import bass2jax

    nc = get_nc()
    bass2jax.install_neuronx_cc_hook()
    part_name = nc.partition_id_tensor.name if nc.partition_id_tensor else None
    in_names, out_names, out_avals, zero_outs = [], [], [], []
    for alloc in nc.m.functions[0].allocations:
        if not isinstance(alloc, mybir.MemoryLocationSet):
            continue
        name = alloc.memorylocations[0].name
        if alloc.kind == "ExternalInput":
            if name != part_name:
                in_names.append(name)
        elif alloc.kind == "ExternalOutput":
            shape = tuple(alloc.tensor_shape)
            dtype = mybir.dt.np(alloc.dtype)
            out_names.append(name)
            out_avals.append(jax.core.ShapedArray(shape, dtype))
            zero_outs.append(np.zeros(shape, dtype))
    all_names = in_names + out_names + ([part_name] if part_name else [])

    def _body(*args):
        operands = list(args)
        if part_name is not None:
            operands.append(bass2jax.partition_id_tensor())
        return tuple(
            bass2jax._bass_exec_p.bind(
                *operands,
                out_avals=tuple(out_avals),
                in_names=tuple(all_names),
                out_names=tuple(out_names),
                lowering_input_output_aliases=(),
                sim_require_finite=True,
                sim_require_nnan=True,
                nc=nc,
            )
        )

    devices = jax.devices()[:NCORES]
    mesh = Mesh(np.array(devices), ("core",))
    specs = (PartitionSpec("core"),) * (len(in_names) + len(out_names))
    out_specs = (PartitionSpec("core"),) * len(out_names)
    sharded = jax.jit(
        shard_map(
            _body, mesh=mesh, in_specs=specs, out_specs=out_specs, check_rep=False
        ),
        keep_unused=True,
    )
    concat_zeros = [
        np.zeros((NCORES * z.shape[0], *z.shape[1:]), z.dtype) for z in zero_outs
    ]
    _RUNNER = (sharded, in_names, concat_zeros)
    return _RUNNER


def kernel(E, px, py, pz):
    arrs = {
        "E": np.ascontiguousarray(np.asarray(E, dtype=np.float32)),
        "px": np.ascontiguousarray(np.asarray(px, dtype=np.float32)),
        "py": np.ascontiguousarray(np.asarray(py, dtype=np.float32)),
        "pz": np.ascontiguousarray(np.asarray(pz, dtype=np.float32)),
    }
    try:
        sharded, in_names, concat_zeros = _get_runner()
        outs = sharded(*[arrs[n] for n in in_names], *concat_zeros)
        return np.asarray(outs[0])
    except Exception:
        # robust fallback: the reference SPMD runner path
        nc = get_nc()
        in_maps = []
        for c in range(NCORES):
            sl = slice(c * ROWS_PER_CORE, (c + 1) * ROWS_PER_CORE)
            in_maps.append({k: v[sl] for k, v in arrs.items()})
        res = bass_utils.run_bass_kernel_spmd(
            nc, in_maps, core_ids=list(range(NCORES))
        )
        return np.concatenate([r["out"] for r in res.results], axis=0)


if __name__ == "__main__":
    rng = np.random.default_rng(0)
    ins = {
        k: rng.standard_normal((TOTAL_ROWS, N), dtype=np.float32)
        for k in ["E", "px", "py", "pz"]
    }
    out = kernel(**ins)
    print("out", out.shape, out.dtype)


# revision 6
# speedup vs baseline: 12.4724x; 1.0701x over previous
"""LBN layer kernel for Trainium2 (8 NeuronCores, pure data parallel).

Inputs: E, px, py, pz each [262144, 16] f32.
Output: [262144, 424] = concat(E, px, py, pz, cross_z, cross_x, cross_y)
where cross(a,b)[r, (i,j)] = a[r,i]*b[r,j] - a[r,j]*b[r,i] over the 120
upper-triangle pairs (i<j), for (a,b) in [(px,py),(py,pz),(pz,px)].

Per-core scheme (32768 rows/core; row r = p*256 + s -> partition p, slot s):
- All tensor compute runs on DVE only: DVE+GpSimd elementwise ops serialize
  on the shared SBUF port (measured), so GpSimd offload is a net loss.
- ACT (scalar engine) pre-builds fp16 operand blocks from the f32 loads:
  A=(px,py,pz), B=(py,pz,px) casts, As/Bs one-element-shifted copies, Ad/Bd
  pair-duplicated copies. These give every product instruction 2-byte dtypes,
  innermost stride-1 APs, and 4B-aligned starts, which keeps the DVE in its
  2x packed perf mode (fp32/broadcast operands would drop it to 1x).
- Per tile: 30 DVE mults write upper/lower pair products a_i*b_j / a_j*b_i
  into an even-padded packed layout; 15 DVE subtracts write the packed triu
  cross features into the fp16 out tile; ACT copies E,px,py,pz passthrough.
- Store: SWDGE cast-DMA (fp16 SBUF -> f32 HBM rows, contiguous per
  partition); loads are plain HWDGE f32.
Measured ~201us/core/iteration vs ~195us for the DMA traffic alone
(63.6MB/core vs ~358GB/s HBM => ~177us roofline).
"""

import numpy as np

import concourse.bass as bass
import concourse.tile as tile
from concourse import bacc, bass_utils, mybir

F32 = mybir.dt.float32
F16 = mybir.dt.float16
N = 16
NPAIR = (N * (N - 1)) // 2  # 120
OUTW = 4 * N + 3 * NPAIR  # 424
TOTAL_ROWS = 262144
NCORES = 8
ROWS_PER_CORE = TOTAL_ROWS // NCORES  # 32768

TRIU_OFF = [0]
for _i in range(1, N):
    TRIU_OFF.append(TRIU_OFF[-1] + (N - _i))

WPAD = [(15 - i) + ((15 - i) & 1) for i in range(15)]  # even-padded widths
POFF = [0]
for _w in WPAD[:-1]:
    POFF.append(POFF[-1] + _w)
PADLEN = POFF[-1] + WPAD[-1]  # 128


def build_kernel(
    nrows=ROWS_PER_CORE,
    nb=16,
    repeat=1,
    share_out=True,
    out_cycle=1,
    xbufs=3,
    fbufs=3,
    ubufs=1,
    obufs=4,
):
    """Build the per-core Bass module.

    nb: row-block (per partition) per tile. repeat: number of full kernel
    executions in one NEFF (used by test.py for amortized HW timing; each
    repetition re-loads inputs from HBM and re-stores the full output).
    share_out: all repetitions write the same DRAM output (identical bytes).
    """
    stripe = nrows // 128
    assert stripe % nb == 0
    nt = stripe // nb

    nc = bacc.Bacc("TRN2", target_bir_lowering=False, debug=False)
    ins = {
        k: nc.dram_tensor(k, [nrows, N], F32, kind="ExternalInput").ap()
        for k in ["E", "px", "py", "pz"]
    }
    nout = min(out_cycle, repeat) if share_out else repeat
    outs = [
        nc.dram_tensor(f"out{r}" if r else "out", [nrows, OUTW], F32,
                       kind="ExternalOutput").ap()
        for r in range(nout)
    ]
    inv = {k: v.rearrange("(p s) f -> p s f", p=128) for k, v in ins.items()}

    B3 = 3 * N  # elems (per nb) of one [3, nb, 16] fp16 block
    OFF_A, OFF_B, OFF_AS, OFF_BS, OFF_AD, OFF_BD = (
        0, B3, 2 * B3, 3 * B3, 4 * B3, 6 * B3)
    XFLEN = 8 * B3  # Ad/Bd are double width

    with tile.TileContext(nc) as tc:
        with (
            tc.tile_pool(name="xt", bufs=xbufs) as xpool,
            tc.tile_pool(name="xf", bufs=fbufs) as fpool,
            tc.tile_pool(name="ul", bufs=ubufs) as ulpool,
            tc.tile_pool(name="ot", bufs=obufs) as opool,
        ):
            for rep in range(repeat):
                outv = outs[rep % nout].rearrange("(p s) f -> p s f", p=128)
                for t in range(nt):
                    # f32 input tile [4, nb, 16] = E,px,py,pz (HWDGE loads)
                    xt = xpool.tile([128, 4 * N * nb], F32, tag="xt")
                    xk = xt[:].rearrange("p (k s i) -> p k s i", s=nb, i=N)
                    for kidx, k in enumerate(["E", "px", "py", "pz"]):
                        nc.sync.dma_start(
                            out=xk[:, kidx],
                            in_=inv[k][:, t * nb : (t + 1) * nb, :],
                        )

                    # fp16 operand tile, built on ACT
                    xf = fpool.tile([128, XFLEN * nb], F16, tag="xf")

                    def blk(off, ln):
                        return xf[:, off * nb : off * nb + ln * nb]

                    A, As = blk(OFF_A, B3), blk(OFF_AS, B3)
                    B, Bs = blk(OFF_B, B3), blk(OFF_BS, B3)
                    Ad, Bd = blk(OFF_AD, 2 * B3), blk(OFF_BD, 2 * B3)

                    nc.scalar.copy(out=A, in_=xt[:, N * nb : 4 * N * nb])
                    nc.scalar.copy(
                        out=B[:, : 2 * N * nb], in_=xt[:, 2 * N * nb : 4 * N * nb]
                    )
                    nc.scalar.copy(
                        out=B[:, 2 * N * nb :], in_=xt[:, N * nb : 2 * N * nb]
                    )
                    # shifted copies, full length: the final element reads the
                    # first element of the following (already written) block
                    nc.scalar.copy(
                        out=As, in_=xf[:, OFF_A * nb + 1 : OFF_A * nb + B3 * nb + 1]
                    )
                    nc.scalar.copy(
                        out=Bs, in_=xf[:, OFF_B * nb + 1 : OFF_B * nb + B3 * nb + 1]
                    )
                    # pair-duplicated copies [fs, i, 2] <- [fs, i] broadcast
                    Av = A.rearrange("p (fs i) -> p fs i", i=N)
                    Bv = B.rearrange("p (fs i) -> p fs i", i=N)
                    Adv = Ad.rearrange("p (fs i two) -> p fs i two", i=N, two=2)
                    Bdv = Bd.rearrange("p (fs i two) -> p fs i two", i=N, two=2)
                    nc.scalar.copy(
                        out=Adv, in_=Av.unsqueeze(3).broadcast_to([128, 3 * nb, N, 2])
                    )
                    nc.scalar.copy(
                        out=Bdv, in_=Bv.unsqueeze(3).broadcast_to([128, 3 * nb, N, 2])
                    )
                    Asv = As.rearrange("p (fs i) -> p fs i", i=N)
                    Bsv = Bs.rearrange("p (fs i) -> p fs i", i=N)

                    # padded u/l product tiles (DVE-only producer+consumer)
                    u3 = ulpool.tile([128, 3 * nb * PADLEN], F16, tag="u3")
                    l3 = ulpool.tile([128, 3 * nb * PADLEN], F16, tag="l3")
                    uq = u3[:].rearrange("p (fs q) -> p fs q", q=PADLEN)
                    lq = l3[:].rearrange("p (fs q) -> p fs q", q=PADLEN)

                    ot = opool.tile([128, nb * OUTW], F16, tag="ot")
                    ov = ot[:].rearrange("p (s c) -> p s c", c=OUTW)
                    pass_dst = ov[:, :, 0 : 4 * N].rearrange(
                        "p s (k i) -> p k s i", i=N
                    )
                    nc.scalar.copy(out=pass_dst, in_=xk[:, 0:4])

                    def pairs(view3, start, np2):
                        return (
                            view3[:, :, start : start + 2 * np2]
                            .rearrange("p fs (q two) -> p fs q two", two=2)
                        )

                    for i in range(15):
                        wp = WPAD[i]
                        q0 = POFF[i]
                        np2 = wp // 2
                        Adp = Adv[:, :, i, :].unsqueeze(2).broadcast_to(
                            [128, 3 * nb, np2, 2]
                        )
                        Bdp = Bdv[:, :, i, :].unsqueeze(2).broadcast_to(
                            [128, 3 * nb, np2, 2]
                        )
                        # window operand starting at element i+1: base block if
                        # i+1 is even, else the shifted block at i (even)
                        if (i + 1) % 2 == 0:
                            bwin = pairs(Bv, i + 1, np2)
                            awin = pairs(Av, i + 1, np2)
                        else:
                            bwin = pairs(Bsv, i, np2)
                            awin = pairs(Asv, i, np2)
                        uo = uq[:, :, q0 : q0 + wp].rearrange(
                            "p fs (q two) -> p fs q two", two=2
                        )
                        lo = lq[:, :, q0 : q0 + wp].rearrange(
                            "p fs (q two) -> p fs q two", two=2
                        )
                        nc.vector.tensor_tensor(uo, Adp, bwin, mybir.AluOpType.mult)
                        nc.vector.tensor_tensor(lo, awin, Bdp, mybir.AluOpType.mult)

                    # per-segment subtract into the packed triu out columns
                    uf = u3[:].rearrange("p (f s q) -> p f s q", s=nb, q=PADLEN)
                    lf = l3[:].rearrange("p (f s q) -> p f s q", s=nb, q=PADLEN)
                    crossv = ov[:, :, 4 * N : OUTW].rearrange(
                        "p s (f q) -> p f s q", q=NPAIR
                    )
                    for i in range(15):
                        w = 15 - i
                        q0 = POFF[i]
                        o0 = TRIU_OFF[i]
                        nc.vector.tensor_tensor(
                            crossv[:, :, :, o0 : o0 + w],
                            uf[:, :, :, q0 : q0 + w],
                            lf[:, :, :, q0 : q0 + w],
                            mybir.AluOpType.subtract,
                        )

                    # SWDGE cast store: fp16 SBUF -> f32 HBM
                    nc.gpsimd.dma_start(
                        out=outv[:, t * nb : (t + 1) * nb, :],
                        in_=ot[:].rearrange("p (s c) -> p s c", c=OUTW),
                    )
    nc.compile()
    return nc


_NC_CACHE = {}


def get_nc(**kw):
    key = tuple(sorted(kw.items()))
    if key not in _NC_CACHE:
        _NC_CACHE[key] = build_kernel(**kw)
    return _NC_CACHE[key]


_RUNNER = None


def _get_runner():
    """Cached jitted shard_map executable over the 8 cores (one compile)."""
    global _RUNNER
    if _RUNNER is not None:
        return _RUNNER
    import jax
    from jax.experimental.shard_map import shard_map
    from jax.sharding import Mesh, PartitionSpec

    from concourse import bass2jax

    nc = get_nc()
    bass2jax.install_neuronx_cc_hook()
    part_name = nc.partition_id_tensor.name if nc.partition_id_tensor else None
    in_names, out_names, out_avals, zero_outs = [], [], [], []
    for alloc in nc.m.functions[0].allocations:
        if not isinstance(alloc, mybir.MemoryLocationSet):
            continue
        name = alloc.memorylocations[0].name
        if alloc.kind == "ExternalInput":
            if name != part_name:
                in_names.append(name)
        elif alloc.kind == "ExternalOutput":
            shape = tuple(alloc.tensor_shape)
            dtype = mybir.dt.np(alloc.dtype)
            out_names.append(name)
            out_avals.append(jax.core.ShapedArray(shape, dtype))
            zero_outs.append(np.zeros(shape, dtype))
    all_names = in_names + out_names + ([part_name] if part_name else [])

    def _body(*args):
        operands = list(args)
        if part_name is not None:
            operands.append(bass2jax.partition_id_tensor())
        return tuple(
            bass2jax._bass_exec_p.bind(
                *operands,
                out_avals=tuple(out_avals),
                in_names=tuple(all_names),
                out_names=tuple(out_names),
                lowering_input_output_aliases=(),
                sim_require_finite=True,
                sim_require_nnan=True,
                nc=nc,
            )
        )

    devices = jax.devices()[:NCORES]
    mesh = Mesh(np.array(devices), ("core",))
    specs = (PartitionSpec("core"),) * (len(in_names) + len(out_names))
    out_specs = (PartitionSpec("core"),) * len(out_names)
    sharded = jax.jit(
        shard_map(
            _body, mesh=mesh, in_specs=specs, out_specs=out_specs, check_rep=False
        ),
        keep_unused=True,
    )
    concat_zeros = [
        np.zeros((NCORES * z.shape[0], *z.shape[1:]), z.dtype) for z in zero_outs
    ]
    _RUNNER = (sharded, in_names, concat_zeros)
    return _RUNNER


def kernel(E, px, py, pz):
    arrs = {
        "E": np.ascontiguousarray(np.asarray(E, dtype=np.float32)),
        "px": np.ascontiguousarray(np.asarray(px, dtype=np.float32)),
        "py": np.ascontiguousarray(np.asarray(py, dtype=np.float32)),
        "pz": np.ascontiguousarray(np.asarray(pz, dtype=np.float32)),
    }
    try:
        sharded, in_names, concat_zeros = _get_runner()
        outs = sharded(*[arrs[n] for n in in_names], *concat_zeros)
        return np.asarray(outs[0])
    except Exception:
        # robust fallback: the reference SPMD runner path
        nc = get_nc()
        in_maps = []
        for c in range(NCORES):
            sl = slice(c * ROWS_PER_CORE, (c + 1) * ROWS_PER_CORE)
            in_maps.append({k: v[sl] for k, v in arrs.items()})
        res = bass_utils.run_bass_kernel_spmd(
            nc, in_maps, core_ids=list(range(NCORES))
        )
        return np.concatenate([r["out"] for r in res.results], axis=0)


if __name__ == "__main__":
    rng = np.random.default_rng(0)
    ins = {
        k: rng.standard_normal((TOTAL_ROWS, N), dtype=np.float32)
        for k in ["E", "px", "py", "pz"]
    }
    out = kernel(**ins)
    print("out", out.shape, out.dtype)
